# revision 11
# baseline (speedup 1.0000x reference)
"""Multi-head causal attention (B=2, S=2048, D=1024, H=16) on 8 trn2 cores.

Sharding: core c -> batch b=c//4, head-group g=c%4 (heads 4g..4g+3).
Each core: Q/K/V projections for its heads from xT[b], causal attention in
transposed layout, row-parallel out-projection partial in bf16. Host sums the
4 partials per batch and adds the bias.

Attention processes a head PAIR per call with row-tiled scores matmuls
(head A on PE rows 0-63, head B on rows 64-127) so the two scores streams
overlap and LDWEIGHTS hides under the other head's matmul. Diagonal
(straddle) blocks compute only the causally-valid q columns
(N = 512/384/256/128 for d = 0..3), so no memset masking is needed — only
the [128,128] triangle multiply on the first 128 valid columns.
"""

import numpy as np

import concourse.bass as bass
import concourse.tile as tile
import concourse.mybir as mybir
from concourse import bacc
from concourse.bass_utils import run_bass_kernel_spmd

B, S, D, H, DH = 2, 2048, 1024, 16, 64
NCORES = 8
HPC = 4          # heads per core
PAIRS = 2        # head pairs per core
QT = 512         # q tile (free dim of scoresT / PV matmuls)
KB = 128         # k block (partition dim of scoresT)
NQT = S // QT    # 4
NKB = S // KB    # 16
DC = D // 128    # 8 contraction chunks for projections
SCALE = 1.0 / np.sqrt(DH)
NWARM = 12       # junk matmuls to pre-warm the PE HAM during input DMA

F32 = mybir.dt.float32
BF = mybir.dt.bfloat16


def _build():
    nc = bacc.Bacc("TRN2", target_bir_lowering=False, debug=False, num_devices=NCORES)

    xT = nc.dram_tensor("xT", [D, S], BF, kind="ExternalInput").ap()
    wq = nc.dram_tensor("wq", [D, HPC * DH], BF, kind="ExternalInput").ap()
    wk = nc.dram_tensor("wk", [D, HPC * DH], BF, kind="ExternalInput").ap()
    wv = nc.dram_tensor("wv", [D, HPC * DH], BF, kind="ExternalInput").ap()
    wo = nc.dram_tensor("wo", [HPC * DH, D], BF, kind="ExternalInput").ap()
    tri = nc.dram_tensor("tri", [KB, KB], BF, kind="ExternalInput").ap()
    out = nc.dram_tensor("out", [S, D], BF, kind="ExternalOutput").ap()

    with tile.TileContext(nc) as tc, \
         tc.tile_pool(name="persist", bufs=1) as persist:
        # ---- persistent tiles ----
        qt_sb = [persist.tile([128, S], BF, name=f"qt{p}", tag=f"qt{p}") for p in range(PAIRS)]
        kt_sb = [persist.tile([128, S], BF, name=f"kt{p}", tag=f"kt{p}") for p in range(PAIRS)]
        # V' tiles: per s-block j, [128, 4*65]; head hl at cols 65*hl, ones col at 65*hl+64
        vt_sb = [persist.tile([128, HPC * (DH + 1)], BF, name=f"vt{j}", tag=f"vt{j}") for j in range(NKB)]
        ctx_sb = [persist.tile([128, S], BF, name=f"ctx{p}", tag=f"ctx{p}") for p in range(PAIRS)]
        wo_sb = [persist.tile([128, D], BF, name=f"wo{p}", tag=f"wo{p}") for p in range(PAIRS)]
        tri_sb = persist.tile([KB, KB], BF, name="tri", tag="tri")
        junk_sb = persist.tile([128, QT], BF, name="junk", tag="junk")

        xts = [persist.tile([128, S], BF, name=f"xts{i}", tag=f"xts{i}") for i in range(DC)]
        wq_sb = [persist.tile([128, HPC * DH], BF, name=f"wq{i}", tag=f"wq{i}") for i in range(DC)]
        wk_sb = [persist.tile([128, HPC * DH], BF, name=f"wk{i}", tag=f"wk{i}") for i in range(DC)]
        wv_sb = [persist.tile([128, HPC * DH], BF, name=f"wv{i}", tag=f"wv{i}") for i in range(DC)]

        nc.gpsimd.memset(junk_sb[:], 0.0)
        # chunk-major DMA order so projection chunk i can chase chunk-i DMAs
        for i in range(DC):
            nc.sync.dma_start(xts[i][:], xT[i * 128:(i + 1) * 128, :])
            nc.sync.dma_start(wq_sb[i][:], wq[i * 128:(i + 1) * 128, :])
            nc.sync.dma_start(wk_sb[i][:], wk[i * 128:(i + 1) * 128, :])
            nc.sync.dma_start(wv_sb[i][:], wv[i * 128:(i + 1) * 128, :])
            if i == 0:
                nc.sync.dma_start(tri_sb[:], tri[:])
        for p in range(PAIRS):
            nc.sync.dma_start(wo_sb[p][:], wo[p * 128:(p + 1) * 128, :])

        # HAM warmup: junk matmuls with no input deps keep the PE busy while
        # the input DMAs land, so the projections start at full clock
        with tc.tile_pool(name="warmps", bufs=1, space="PSUM") as warmps:
            wp = warmps.tile([128, QT], F32, name="wp", tag="wp")
            for _ in range(NWARM):
                nc.tensor.matmul(wp[:], junk_sb[:, 0:128], junk_sb[:],
                                 start=True, stop=True)

        def proj_qk_chunked(p, pool):
            """q/k projection for pair p, D-chunk-outer so matmuls chase the
            xT DMAs chunk by chunk. Holds 8 psum banks."""
            qps = [pool.tile([128, QT], F32, name=f"qps{st}", tag=f"qk{st}") for st in range(NQT)]
            kps = [pool.tile([128, QT], F32, name=f"kps{st}", tag=f"qk{4 + st}") for st in range(NQT)]
            for i in range(DC):
                for st in range(NQT):
                    nc.tensor.matmul(
                        qps[st][:], wq_sb[i][:, p * 128:(p + 1) * 128],
                        xts[i][:, st * QT:(st + 1) * QT],
                        start=(i == 0), stop=(i == DC - 1))
                for st in range(NQT):
                    nc.tensor.matmul(
                        kps[st][:], wk_sb[i][:, p * 128:(p + 1) * 128],
                        xts[i][:, st * QT:(st + 1) * QT],
                        start=(i == 0), stop=(i == DC - 1))
            for st in range(NQT):
                nc.vector.tensor_copy(qt_sb[p][:, st * QT:(st + 1) * QT], qps[st][:])
                nc.vector.tensor_copy(kt_sb[p][:, st * QT:(st + 1) * QT], kps[st][:])

        def proj_qk_seq(p, pool):
            """q/k projection, sequential psum (2 banks) — overlaps with
            attention of the other pair."""
            for st in range(NQT):
                qp = pool.tile([128, QT], F32, name="qp", tag="qkseq")
                for i in range(DC):
                    nc.tensor.matmul(
                        qp[:], wq_sb[i][:, p * 128:(p + 1) * 128],
                        xts[i][:, st * QT:(st + 1) * QT],
                        start=(i == 0), stop=(i == DC - 1))
                nc.vector.tensor_copy(qt_sb[p][:, st * QT:(st + 1) * QT], qp[:])
                kp = pool.tile([128, QT], F32, name="kp", tag="qkseq")
                for i in range(DC):
                    nc.tensor.matmul(
                        kp[:], wk_sb[i][:, p * 128:(p + 1) * 128],
                        xts[i][:, st * QT:(st + 1) * QT],
                        start=(i == 0), stop=(i == DC - 1))
                nc.vector.tensor_copy(kt_sb[p][:, st * QT:(st + 1) * QT], kp[:])

        def attention_pair(p, qt_i, scps, att, ctxps, attsm):
            """Both heads of pair p for q-tile qt_i. Scores row-tiled: head A
            contracts on PE rows 0-63, head B on rows 64-127 (base_partition
            derives tile_position), so the two streams overlap and LDW hides.
            Straddle blocks d=0..3 compute only q columns >= 128*d."""
            q0 = qt_i * QT
            nkb = 4 * qt_i + 4
            cps = [ctxps.tile([DH + 1, QT], F32, name=f"cps{h}", tag=f"cps{h}")
                   for h in range(2)]
            for kb in range(nkb):
                d = kb - 4 * qt_i
                if d < 0:
                    qlo, n = 0, QT
                else:
                    qlo, n = 128 * d, QT - 128 * d
                # head B always in the second psum bank: two concurrent
                # row-tiled matmuls writing one bank crashes the runtime
                boff = QT
                sp = scps.tile([128, 2 * QT], F32, name="sp", tag="sp")
                pt = att.tile([128, 2 * QT], BF, name="pt", tag="pt")
                for h in range(2):
                    r0 = h * 64
                    off = h * boff
                    nc.tensor.matmul(
                        sp[:, off:off + n],
                        kt_sb[p][r0:r0 + 64, kb * KB:(kb + 1) * KB],
                        qt_sb[p][r0:r0 + 64, q0 + qlo:q0 + QT],
                        start=True, stop=True)
                if boff == n or n == QT:
                    nc.scalar.activation(
                        pt[:, 0:boff + n], sp[:, 0:boff + n],
                        mybir.ActivationFunctionType.Exp, scale=float(SCALE))
                else:
                    for h in range(2):
                        off = h * boff
                        nc.scalar.activation(
                            pt[:, off:off + n], sp[:, off:off + n],
                            mybir.ActivationFunctionType.Exp, scale=float(SCALE))
                if d >= 0:
                    # diagonal triangle on the first 128 valid columns
                    nc.vector.tensor_mul(pt[:, 0:KB], pt[:, 0:KB], tri_sb[:])
                    nc.vector.tensor_mul(
                        pt[:, boff:boff + KB], pt[:, boff:boff + KB], tri_sb[:])
                for h in range(2):
                    hl = 2 * p + h
                    off = h * boff
                    nc.tensor.matmul(
                        cps[h][:, qlo:QT],
                        vt_sb[kb][:, hl * (DH + 1):(hl + 1) * (DH + 1)],
                        pt[:, off:off + n],
                        start=(kb == 0), stop=(kb == nkb - 1))
            for h in range(2):
                r0 = h * 64
                # r = 1/l as exp(-ln(l)) on ACT: ~0.9us vs 3.35us DVE reciprocal
                t_sb = attsm.tile([1, QT], F32, name="t_sb", tag="t")
                nc.scalar.activation(t_sb[:], cps[h][DH:DH + 1, :],
                                     mybir.ActivationFunctionType.Ln)
                r_sb = attsm.tile([1, QT], F32, name="r_sb", tag="r")
                nc.scalar.activation(r_sb[:], t_sb[:],
                                     mybir.ActivationFunctionType.Exp, scale=-1.0)
                rb = attsm.tile([64, QT], F32, name="rb", tag="rb")
                nc.gpsimd.partition_broadcast(rb[:], r_sb[:])
                nc.vector.tensor_mul(
                    ctx_sb[p][r0:r0 + 64, q0:q0 + QT], cps[h][0:DH, :], rb[:])

        def outproj(qt_i, ph3ps, ph3sb):
            """partial out-projection rows for one q tile; bias is added on
            the host after the cross-core partial sum."""
            for qb in range(qt_i * 4, qt_i * 4 + 4):
                os_ = ph3sb.tile([128, D], BF, name="os", tag="os")
                for nh in range(2):
                    op = ph3ps.tile([128, 512], F32, name="op", tag="op")
                    for p in range(PAIRS):
                        nc.tensor.matmul(
                            op[:], ctx_sb[p][:, qb * 128:(qb + 1) * 128],
                            wo_sb[p][:, nh * 512:(nh + 1) * 512],
                            start=(p == 0), stop=(p == PAIRS - 1))
                    nc.vector.tensor_copy(os_[:, nh * 512:(nh + 1) * 512], op[:])
                nc.sync.dma_start(out[qb * 128:(qb + 1) * 128, :], os_[:])

        # phase A: q/k pair 0, chunk-pipelined against the input DMAs
        with tc.tile_pool(name="qk0ps", bufs=1, space="PSUM") as qk0ps:
            proj_qk_chunked(0, qk0ps)

        # phase B onwards: V (2 psum banks) + attention pools (6 banks)
        with tc.tile_pool(name="att", bufs=4) as att, \
             tc.tile_pool(name="attsm", bufs=4) as attsm, \
             tc.tile_pool(name="scps", bufs=2, space="PSUM") as scps, \
             tc.tile_pool(name="ctxps", bufs=1, space="PSUM") as ctxps:

            with tc.tile_pool(name="vps", bufs=2, space="PSUM") as vps:
                for j in range(NKB):
                    vp = vps.tile([128, HPC * DH], F32, name="vp", tag="vp")
                    for i in range(DC):
                        nc.tensor.matmul(
                            vp[:], xts[i][:, j * 128:(j + 1) * 128], wv_sb[i][:],
                            start=(i == 0), stop=(i == DC - 1))
                    vt_view = vt_sb[j].rearrange("p (h e) -> p h e", h=HPC)
                    nc.vector.tensor_copy(
                        vt_view[:, :, 0:DH], vp.rearrange("p (h e) -> p h e", h=HPC))
                    nc.gpsimd.memset(vt_view[:, :, DH:DH + 1], 1.0)

            # pair-0 attention (starts as soon as early vt tiles land);
            # q/k pair-1 proj fills PE gaps once the vps banks free up
            with tc.tile_pool(name="qk1ps", bufs=2, space="PSUM") as qk1ps:
                for qt_i in range(NQT):
                    attention_pair(0, qt_i, scps, att, ctxps, attsm)
                proj_qk_seq(1, qk1ps)

            # pair-1 attention, out-projection interleaved per finished q tile
            with tc.tile_pool(name="ph3ps", bufs=2, space="PSUM") as ph3ps, \
                 tc.tile_pool(name="ph3sb", bufs=3) as ph3sb:
                for qt_i in range(NQT):
                    if qt_i > 0:
                        outproj(qt_i - 1, ph3ps, ph3sb)
                    attention_pair(1, qt_i, scps, att, ctxps, attsm)
                outproj(NQT - 1, ph3ps, ph3sb)

    nc.compile()
    return nc


_NC = None
PROFILE = False
TRACE_CORES = (0,)
LAST_RESULT = None


def _get_nc():
    global _NC
    if _NC is None:
        _NC = _build()
    return _NC


def kernel(x, Wq, Wk, Wv, Wo, bo):
    x = np.asarray(x, dtype=np.float32)
    Wq = np.asarray(Wq, dtype=np.float32)
    Wk = np.asarray(Wk, dtype=np.float32)
    Wv = np.asarray(Wv, dtype=np.float32)
    Wo = np.asarray(Wo, dtype=np.float32)
    bo = np.asarray(bo, dtype=np.float32)

    nc = _get_nc()

    in_maps = _prepare_in_maps(x, Wq, Wk, Wv, Wo)

    global LAST_RESULT
    kw = {}
    if PROFILE:
        kw = dict(trace=True, trace_cores=list(TRACE_CORES))
    res = run_bass_kernel_spmd(nc, in_maps, core_ids=list(range(NCORES)), **kw)
    LAST_RESULT = res

    out = np.zeros((B, S, D), np.float32)
    for c in range(NCORES):
        b = c // 4
        out[b] += np.asarray(res.results[c]["out"], dtype=np.float32)
    out += bo[None, None, :]
    return out


def _prepare_in_maps(x, Wq, Wk, Wv, Wo):
    kk = np.arange(KB)[:, None]
    qq = np.arange(KB)[None, :]
    import ml_dtypes
    tri = (kk <= qq).astype(ml_dtypes.bfloat16)

    bf16 = ml_dtypes.bfloat16
    xTs = [np.ascontiguousarray(x[b].T).astype(bf16) for b in range(B)]

    in_maps = []
    for c in range(NCORES):
        b, g = divmod(c, 4)
        cs = slice(g * HPC * DH, (g + 1) * HPC * DH)
        in_maps.append({
            "xT": xTs[b],
            "wq": np.ascontiguousarray(Wq[:, cs]).astype(bf16),
            "wk": np.ascontiguousarray(Wk[:, cs]).astype(bf16),
            "wv": np.ascontiguousarray(Wv[:, cs]).astype(bf16),
            "wo": np.ascontiguousarray(Wo[cs, :]).astype(bf16),
            "tri": tri,
        })
    return in_maps


# revision 19
# speedup vs baseline: 1.0349x; 1.0349x over previous
"""Multi-head causal attention (B=2, S=2048, D=1024, H=16) on 8 trn2 cores.

Sharding: core c -> batch b=c//4, head-group g=c%4 (heads 4g..4g+3).
Each core: Q/K/V projections for its heads from xT[b], causal attention in
transposed layout, row-parallel out-projection partial in bf16. Host sums the
4 partials per batch and adds the bias.

Attention processes a head PAIR per call with row-tiled scores matmuls
(head A on PE rows 0-63, head B on rows 64-127) so the two scores streams
overlap and LDWEIGHTS hides under the other head's matmul. Diagonal
(straddle) blocks compute only the causally-valid q columns
(N = 512/384/256/128 for d = 0..3), so no memset masking is needed — only
the [128,128] triangle multiply on the first 128 valid columns.
"""

import numpy as np

import concourse.bass as bass
import concourse.tile as tile
import concourse.mybir as mybir
from concourse import bacc
from concourse.bass_utils import run_bass_kernel_spmd

B, S, D, H, DH = 2, 2048, 1024, 16, 64
NCORES = 8
HPC = 4          # heads per core
PAIRS = 2        # head pairs per core
QT = 512         # q tile (free dim of scoresT / PV matmuls)
KB = 128         # k block (partition dim of scoresT)
NQT = S // QT    # 4
NKB = S // KB    # 16
DC = D // 128    # 8 contraction chunks for projections
SCALE = 1.0 / np.sqrt(DH)
NWARM = 12       # junk matmuls to pre-warm the PE HAM during input DMA

F32 = mybir.dt.float32
BF = mybir.dt.bfloat16


def _build():
    nc = bacc.Bacc("TRN2", target_bir_lowering=False, debug=False, num_devices=NCORES)

    xT = nc.dram_tensor("xT", [D, S], BF, kind="ExternalInput").ap()
    wq = nc.dram_tensor("wq", [D, HPC * DH], BF, kind="ExternalInput").ap()
    wk = nc.dram_tensor("wk", [D, HPC * DH], BF, kind="ExternalInput").ap()
    wv = nc.dram_tensor("wv", [D, HPC * DH], BF, kind="ExternalInput").ap()
    wo = nc.dram_tensor("wo", [HPC * DH, D], BF, kind="ExternalInput").ap()
    tri = nc.dram_tensor("tri", [KB, KB], BF, kind="ExternalInput").ap()
    out = nc.dram_tensor("out", [S, D], BF, kind="ExternalOutput").ap()

    with tile.TileContext(nc) as tc, \
         tc.tile_pool(name="persist", bufs=1) as persist:
        # ---- persistent tiles ----
        qt_sb = [persist.tile([128, S], BF, name=f"qt{p}", tag=f"qt{p}") for p in range(PAIRS)]
        kt_sb = [persist.tile([128, S], BF, name=f"kt{p}", tag=f"kt{p}") for p in range(PAIRS)]
        # V' tiles: per s-block j, [128, 4*65]; head hl at cols 65*hl, ones col at 65*hl+64
        vt_sb = [persist.tile([128, HPC * (DH + 1)], BF, name=f"vt{j}", tag=f"vt{j}") for j in range(NKB)]
        ctx_sb = [persist.tile([128, S], BF, name=f"ctx{p}", tag=f"ctx{p}") for p in range(PAIRS)]
        wo_sb = [persist.tile([128, D], BF, name=f"wo{p}", tag=f"wo{p}") for p in range(PAIRS)]
        tri_sb = persist.tile([KB, KB], BF, name="tri", tag="tri")
        junk_sb = persist.tile([128, QT], BF, name="junk", tag="junk")

        xts = [persist.tile([128, S], BF, name=f"xts{i}", tag=f"xts{i}") for i in range(DC)]
        wq_sb = [persist.tile([128, HPC * DH], BF, name=f"wq{i}", tag=f"wq{i}") for i in range(DC)]
        wk_sb = [persist.tile([128, HPC * DH], BF, name=f"wk{i}", tag=f"wk{i}") for i in range(DC)]
        wv_sb = [persist.tile([128, HPC * DH], BF, name=f"wv{i}", tag=f"wv{i}") for i in range(DC)]

        nc.gpsimd.memset(junk_sb[:], 0.0)
        # chunk-major DMA order so projection chunk i can chase chunk-i DMAs
        for i in range(DC):
            nc.sync.dma_start(xts[i][:], xT[i * 128:(i + 1) * 128, :])
            nc.sync.dma_start(wq_sb[i][:], wq[i * 128:(i + 1) * 128, :])
            nc.sync.dma_start(wk_sb[i][:], wk[i * 128:(i + 1) * 128, :])
            nc.sync.dma_start(wv_sb[i][:], wv[i * 128:(i + 1) * 128, :])
            if i == 0:
                nc.sync.dma_start(tri_sb[:], tri[:])
        for p in range(PAIRS):
            nc.sync.dma_start(wo_sb[p][:], wo[p * 128:(p + 1) * 128, :])

        # HAM warmup: junk matmuls with no input deps keep the PE busy while
        # the input DMAs land, so the projections start at full clock
        with tc.tile_pool(name="warmps", bufs=1, space="PSUM") as warmps:
            wp = warmps.tile([128, QT], F32, name="wp", tag="wp")
            for _ in range(NWARM):
                nc.tensor.matmul(wp[:], junk_sb[:, 0:128], junk_sb[:],
                                 start=True, stop=True)

        def proj_qk_chunked(p, pool):
            """q/k projection for pair p, D-chunk-outer so matmuls chase the
            xT DMAs chunk by chunk. Holds 8 psum banks."""
            qps = [pool.tile([128, QT], F32, name=f"qps{st}", tag=f"qk{st}") for st in range(NQT)]
            kps = [pool.tile([128, QT], F32, name=f"kps{st}", tag=f"qk{4 + st}") for st in range(NQT)]
            for i in range(DC):
                for st in range(NQT):
                    nc.tensor.matmul(
                        qps[st][:], wq_sb[i][:, p * 128:(p + 1) * 128],
                        xts[i][:, st * QT:(st + 1) * QT],
                        start=(i == 0), stop=(i == DC - 1))
                for st in range(NQT):
                    nc.tensor.matmul(
                        kps[st][:], wk_sb[i][:, p * 128:(p + 1) * 128],
                        xts[i][:, st * QT:(st + 1) * QT],
                        start=(i == 0), stop=(i == DC - 1))
            for st in range(NQT):
                nc.vector.tensor_copy(qt_sb[p][:, st * QT:(st + 1) * QT], qps[st][:])
                nc.vector.tensor_copy(kt_sb[p][:, st * QT:(st + 1) * QT], kps[st][:])

        def proj_qk_seq(p, pool):
            """q/k projection, sequential psum (2 banks) — overlaps with
            attention of the other pair."""
            for st in range(NQT):
                qp = pool.tile([128, QT], F32, name="qp", tag="qkseq")
                for i in range(DC):
                    nc.tensor.matmul(
                        qp[:], wq_sb[i][:, p * 128:(p + 1) * 128],
                        xts[i][:, st * QT:(st + 1) * QT],
                        start=(i == 0), stop=(i == DC - 1))
                nc.vector.tensor_copy(qt_sb[p][:, st * QT:(st + 1) * QT], qp[:])
                kp = pool.tile([128, QT], F32, name="kp", tag="qkseq")
                for i in range(DC):
                    nc.tensor.matmul(
                        kp[:], wk_sb[i][:, p * 128:(p + 1) * 128],
                        xts[i][:, st * QT:(st + 1) * QT],
                        start=(i == 0), stop=(i == DC - 1))
                nc.vector.tensor_copy(kt_sb[p][:, st * QT:(st + 1) * QT], kp[:])

        def attention_pair(p, qt_i, scps, att, ctxps, attsm):
            """Both heads of pair p for q-tile qt_i. Scores row-tiled: head A
            contracts on PE rows 0-63, head B on rows 64-127 (base_partition
            derives tile_position), so the two streams overlap and LDW hides.
            Straddle blocks d=0..3 compute only q columns >= 128*d."""
            q0 = qt_i * QT
            nkb = 4 * qt_i + 4
            cps = [ctxps.tile([DH + 1, QT], F32, name=f"cps{h}", tag=f"cps{h}")
                   for h in range(2)]
            for kb in range(nkb):
                d = kb - 4 * qt_i
                if d < 0:
                    qlo, n = 0, QT
                else:
                    qlo, n = 128 * d, QT - 128 * d
                # head B always in the second psum bank: two concurrent
                # row-tiled matmuls writing one bank crashes the runtime
                boff = QT
                sp = scps.tile([128, 2 * QT], F32, name="sp", tag="sp")
                pt = att.tile([128, 2 * QT], BF, name="pt", tag="pt")
                for h in range(2):
                    r0 = h * 64
                    off = h * boff
                    nc.tensor.matmul(
                        sp[:, off:off + n],
                        kt_sb[p][r0:r0 + 64, kb * KB:(kb + 1) * KB],
                        qt_sb[p][r0:r0 + 64, q0 + qlo:q0 + QT],
                        start=True, stop=True)
                if boff == n or n == QT:
                    nc.scalar.activation(
                        pt[:, 0:boff + n], sp[:, 0:boff + n],
                        mybir.ActivationFunctionType.Exp, scale=float(SCALE))
                else:
                    for h in range(2):
                        off = h * boff
                        nc.scalar.activation(
                            pt[:, off:off + n], sp[:, off:off + n],
                            mybir.ActivationFunctionType.Exp, scale=float(SCALE))
                if d >= 0:
                    # diagonal triangle on the first 128 valid columns
                    nc.vector.tensor_mul(pt[:, 0:KB], pt[:, 0:KB], tri_sb[:])
                    nc.vector.tensor_mul(
                        pt[:, boff:boff + KB], pt[:, boff:boff + KB], tri_sb[:])
                for h in range(2):
                    hl = 2 * p + h
                    off = h * boff
                    nc.tensor.matmul(
                        cps[h][:, qlo:QT],
                        vt_sb[kb][:, hl * (DH + 1):(hl + 1) * (DH + 1)],
                        pt[:, off:off + n],
                        start=(kb == 0), stop=(kb == nkb - 1))
            for h in range(2):
                r0 = h * 64
                # 1/l on DVE (no ACT table thrash: Ln on ACT forces a 1.3us
                # ACT_TABLE_LOAD on every Exp<->Ln switch)
                r_sb = attsm.tile([1, QT], F32, name="r_sb", tag="r")
                nc.vector.reciprocal(r_sb[:], cps[h][DH:DH + 1, :])
                rb = attsm.tile([64, QT], F32, name="rb", tag="rb")
                nc.gpsimd.partition_broadcast(rb[:], r_sb[:])
                nc.vector.tensor_mul(
                    ctx_sb[p][r0:r0 + 64, q0:q0 + QT], cps[h][0:DH, :], rb[:])

        def outproj(qt_i, ph3ps, ph3sb):
            """partial out-projection rows for one q tile; bias is added on
            the host after the cross-core partial sum."""
            for qb in range(qt_i * 4, qt_i * 4 + 4):
                os_ = ph3sb.tile([128, D], BF, name="os", tag="os")
                for nh in range(2):
                    op = ph3ps.tile([128, 512], F32, name="op", tag="op")
                    for p in range(PAIRS):
                        nc.tensor.matmul(
                            op[:], ctx_sb[p][:, qb * 128:(qb + 1) * 128],
                            wo_sb[p][:, nh * 512:(nh + 1) * 512],
                            start=(p == 0), stop=(p == PAIRS - 1))
                    nc.vector.tensor_copy(os_[:, nh * 512:(nh + 1) * 512], op[:])
                nc.sync.dma_start(out[qb * 128:(qb + 1) * 128, :], os_[:])

        # phase A: q/k pair 0, chunk-pipelined against the input DMAs
        with tc.tile_pool(name="qk0ps", bufs=1, space="PSUM") as qk0ps:
            proj_qk_chunked(0, qk0ps)

        # phase B onwards: V (2 psum banks) + attention pools (6 banks)
        with tc.tile_pool(name="att", bufs=4) as att, \
             tc.tile_pool(name="attsm", bufs=4) as attsm, \
             tc.tile_pool(name="scps", bufs=2, space="PSUM") as scps, \
             tc.tile_pool(name="ctxps", bufs=1, space="PSUM") as ctxps:

            def v_blocks(lo, hi, vps):
                for j in range(lo, hi):
                    vp = vps.tile([128, HPC * DH], F32, name="vp", tag="vp")
                    for i in range(DC):
                        nc.tensor.matmul(
                            vp[:], xts[i][:, j * 128:(j + 1) * 128], wv_sb[i][:],
                            start=(i == 0), stop=(i == DC - 1))
                    vt_view = vt_sb[j].rearrange("p (h e) -> p h e", h=HPC)
                    nc.vector.tensor_copy(
                        vt_view[:, :, 0:DH], vp.rearrange("p (h e) -> p h e", h=HPC))
                    nc.gpsimd.memset(vt_view[:, :, DH:DH + 1], 1.0)

            # pair-0 attention with the V blocks for q-tile t+1 emitted between
            # q-tiles: independent PE filler positioned at the cps-reuse stalls
            with tc.tile_pool(name="vps", bufs=2, space="PSUM") as vps:
                v_blocks(0, 4, vps)
                for qt_i in range(NQT):
                    attention_pair(0, qt_i, scps, att, ctxps, attsm)
                    if qt_i < NQT - 1:
                        v_blocks(4 * qt_i + 4, 4 * qt_i + 8, vps)

            # q/k pair 1 fills the tail of pair-0 attention
            with tc.tile_pool(name="qk1ps", bufs=2, space="PSUM") as qk1ps:
                proj_qk_seq(1, qk1ps)

            # pair-1 attention; outproj(qt-1) is fully ready by the end of
            # attention qt, so it fills the qt -> qt+1 boundary stall
            with tc.tile_pool(name="ph3ps", bufs=2, space="PSUM") as ph3ps, \
                 tc.tile_pool(name="ph3sb", bufs=3) as ph3sb:
                for qt_i in range(NQT):
                    attention_pair(1, qt_i, scps, att, ctxps, attsm)
                    if qt_i > 0:
                        outproj(qt_i - 1, ph3ps, ph3sb)
                outproj(NQT - 1, ph3ps, ph3sb)

    nc.compile()
    return nc


_NC = None
PROFILE = False
TRACE_CORES = (0,)
LAST_RESULT = None


def _get_nc():
    global _NC
    if _NC is None:
        _NC = _build()
    return _NC


def kernel(x, Wq, Wk, Wv, Wo, bo):
    x = np.asarray(x, dtype=np.float32)
    Wq = np.asarray(Wq, dtype=np.float32)
    Wk = np.asarray(Wk, dtype=np.float32)
    Wv = np.asarray(Wv, dtype=np.float32)
    Wo = np.asarray(Wo, dtype=np.float32)
    bo = np.asarray(bo, dtype=np.float32)

    nc = _get_nc()

    in_maps = _prepare_in_maps(x, Wq, Wk, Wv, Wo)

    global LAST_RESULT
    kw = {}
    if PROFILE:
        kw = dict(trace=True, trace_cores=list(TRACE_CORES))
    res = run_bass_kernel_spmd(nc, in_maps, core_ids=list(range(NCORES)), **kw)
    LAST_RESULT = res

    out = np.zeros((B, S, D), np.float32)
    for c in range(NCORES):
        b = c // 4
        out[b] += np.asarray(res.results[c]["out"], dtype=np.float32)
    out += bo[None, None, :]
    return out


def _prepare_in_maps(x, Wq, Wk, Wv, Wo):
    kk = np.arange(KB)[:, None]
    qq = np.arange(KB)[None, :]
    import ml_dtypes
    tri = (kk <= qq).astype(ml_dtypes.bfloat16)

    bf16 = ml_dtypes.bfloat16
    xTs = [np.ascontiguousarray(x[b].T).astype(bf16) for b in range(B)]

    in_maps = []
    for c in range(NCORES):
        b, g = divmod(c, 4)
        cs = slice(g * HPC * DH, (g + 1) * HPC * DH)
        in_maps.append({
            "xT": xTs[b],
            "wq": np.ascontiguousarray(Wq[:, cs]).astype(bf16),
            "wk": np.ascontiguousarray(Wk[:, cs]).astype(bf16),
            "wv": np.ascontiguousarray(Wv[:, cs]).astype(bf16),
            "wo": np.ascontiguousarray(Wo[cs, :]).astype(bf16),
            "tri": tri,
        })
    return in_maps


# revision 20
# speedup vs baseline: 1.0987x; 1.0617x over previous
"""Multi-head causal attention (B=2, S=2048, D=1024, H=16) on 8 trn2 cores.

Sharding: core c -> batch b=c//4, head-group g=c%4 (heads 4g..4g+3).
Each core: Q/K/V projections for its heads from xT[b], causal attention in
transposed layout, row-parallel out-projection partial in bf16. Host sums the
4 partials per batch and adds the bias.

Attention processes a head PAIR per call with row-tiled scores matmuls
(head A on PE rows 0-63, head B on rows 64-127) so the two scores streams
overlap and LDWEIGHTS hides under the other head's matmul. Diagonal
(straddle) blocks compute only the causally-valid q columns
(N = 512/384/256/128 for d = 0..3), so no memset masking is needed — only
the [128,128] triangle multiply on the first 128 valid columns.
"""

import numpy as np

import concourse.bass as bass
import concourse.tile as tile
import concourse.mybir as mybir
from concourse import bacc
from concourse.bass_utils import run_bass_kernel_spmd

B, S, D, H, DH = 2, 2048, 1024, 16, 64
NCORES = 8
HPC = 4          # heads per core
PAIRS = 2        # head pairs per core
QT = 512         # q tile (free dim of scoresT / PV matmuls)
KB = 128         # k block (partition dim of scoresT)
NQT = S // QT    # 4
NKB = S // KB    # 16
DC = D // 128    # 8 contraction chunks for projections
SCALE = 1.0 / np.sqrt(DH)
NWARM = 12       # junk matmuls to pre-warm the PE HAM during input DMA

F32 = mybir.dt.float32
BF = mybir.dt.bfloat16


def _build():
    nc = bacc.Bacc("TRN2", target_bir_lowering=False, debug=False, num_devices=NCORES)

    xT = nc.dram_tensor("xT", [D, S], BF, kind="ExternalInput").ap()
    wq = nc.dram_tensor("wq", [D, HPC * DH], BF, kind="ExternalInput").ap()
    wk = nc.dram_tensor("wk", [D, HPC * DH], BF, kind="ExternalInput").ap()
    wv = nc.dram_tensor("wv", [D, HPC * DH], BF, kind="ExternalInput").ap()
    wo = nc.dram_tensor("wo", [HPC * DH, D], BF, kind="ExternalInput").ap()
    tri = nc.dram_tensor("tri", [KB, KB], BF, kind="ExternalInput").ap()
    out = nc.dram_tensor("out", [S, D], BF, kind="ExternalOutput").ap()

    with tile.TileContext(nc) as tc, \
         tc.tile_pool(name="persist", bufs=1) as persist:
        # ---- persistent tiles ----
        qt_sb = [persist.tile([128, S], BF, name=f"qt{p}", tag=f"qt{p}") for p in range(PAIRS)]
        kt_sb = [persist.tile([128, S], BF, name=f"kt{p}", tag=f"kt{p}") for p in range(PAIRS)]
        # V' tiles: per s-block j, [128, 4*65]; head hl at cols 65*hl, ones col at 65*hl+64
        vt_sb = [persist.tile([128, HPC * (DH + 1)], BF, name=f"vt{j}", tag=f"vt{j}") for j in range(NKB)]
        ctx_sb = [persist.tile([128, S], BF, name=f"ctx{p}", tag=f"ctx{p}") for p in range(PAIRS)]
        wo_sb = [persist.tile([128, D], BF, name=f"wo{p}", tag=f"wo{p}") for p in range(PAIRS)]
        tri_sb = persist.tile([KB, KB], BF, name="tri", tag="tri")
        junk_sb = persist.tile([128, QT], BF, name="junk", tag="junk")

        xts = [persist.tile([128, S], BF, name=f"xts{i}", tag=f"xts{i}") for i in range(DC)]
        wq_sb = [persist.tile([128, HPC * DH], BF, name=f"wq{i}", tag=f"wq{i}") for i in range(DC)]
        wk_sb = [persist.tile([128, HPC * DH], BF, name=f"wk{i}", tag=f"wk{i}") for i in range(DC)]
        wv_sb = [persist.tile([128, HPC * DH], BF, name=f"wv{i}", tag=f"wv{i}") for i in range(DC)]

        nc.gpsimd.memset(junk_sb[:], 0.0)
        # chunk-major DMA order so projection chunk i can chase chunk-i DMAs
        for i in range(DC):
            nc.sync.dma_start(xts[i][:], xT[i * 128:(i + 1) * 128, :])
            nc.sync.dma_start(wq_sb[i][:], wq[i * 128:(i + 1) * 128, :])
            nc.sync.dma_start(wk_sb[i][:], wk[i * 128:(i + 1) * 128, :])
            nc.sync.dma_start(wv_sb[i][:], wv[i * 128:(i + 1) * 128, :])
            if i == 0:
                nc.sync.dma_start(tri_sb[:], tri[:])
        for p in range(PAIRS):
            nc.sync.dma_start(wo_sb[p][:], wo[p * 128:(p + 1) * 128, :])

        # HAM warmup: junk matmuls with no input deps keep the PE busy while
        # the input DMAs land, so the projections start at full clock
        with tc.tile_pool(name="warmps", bufs=1, space="PSUM") as warmps:
            wp = warmps.tile([128, QT], F32, name="wp", tag="wp")
            for _ in range(NWARM):
                nc.tensor.matmul(wp[:], junk_sb[:, 0:128], junk_sb[:],
                                 start=True, stop=True)

        def proj_qk_chunked(p, pool):
            """q/k projection for pair p, D-chunk-outer so matmuls chase the
            xT DMAs chunk by chunk. Holds 8 psum banks."""
            qps = [pool.tile([128, QT], F32, name=f"qps{st}", tag=f"qk{st}") for st in range(NQT)]
            kps = [pool.tile([128, QT], F32, name=f"kps{st}", tag=f"qk{4 + st}") for st in range(NQT)]
            for i in range(DC):
                for st in range(NQT):
                    nc.tensor.matmul(
                        qps[st][:], wq_sb[i][:, p * 128:(p + 1) * 128],
                        xts[i][:, st * QT:(st + 1) * QT],
                        start=(i == 0), stop=(i == DC - 1))
                for st in range(NQT):
                    nc.tensor.matmul(
                        kps[st][:], wk_sb[i][:, p * 128:(p + 1) * 128],
                        xts[i][:, st * QT:(st + 1) * QT],
                        start=(i == 0), stop=(i == DC - 1))
            for st in range(NQT):
                nc.vector.tensor_copy(qt_sb[p][:, st * QT:(st + 1) * QT], qps[st][:])
                nc.vector.tensor_copy(kt_sb[p][:, st * QT:(st + 1) * QT], kps[st][:])

        def proj_qk_seq(p, pool):
            """q/k projection, sequential psum (2 banks) — overlaps with
            attention of the other pair."""
            for st in range(NQT):
                qp = pool.tile([128, QT], F32, name="qp", tag="qkseq")
                for i in range(DC):
                    nc.tensor.matmul(
                        qp[:], wq_sb[i][:, p * 128:(p + 1) * 128],
                        xts[i][:, st * QT:(st + 1) * QT],
                        start=(i == 0), stop=(i == DC - 1))
                nc.vector.tensor_copy(qt_sb[p][:, st * QT:(st + 1) * QT], qp[:])
                kp = pool.tile([128, QT], F32, name="kp", tag="qkseq")
                for i in range(DC):
                    nc.tensor.matmul(
                        kp[:], wk_sb[i][:, p * 128:(p + 1) * 128],
                        xts[i][:, st * QT:(st + 1) * QT],
                        start=(i == 0), stop=(i == DC - 1))
                nc.vector.tensor_copy(kt_sb[p][:, st * QT:(st + 1) * QT], kp[:])

        def attention_pair(p, qt_i, scps, att, ctxps, attsm):
            """Both heads of pair p for q-tile qt_i. Scores row-tiled: head A
            contracts on PE rows 0-63, head B on rows 64-127 (base_partition
            derives tile_position), so the two streams overlap and LDW hides.
            Straddle blocks d=0..3 compute only q columns >= 128*d."""
            q0 = qt_i * QT
            nkb = 4 * qt_i + 4
            cps = [ctxps.tile([DH + 1, QT], F32, name=f"cps{h}", tag=f"cps{h}")
                   for h in range(2)]
            for kb in range(nkb):
                d = kb - 4 * qt_i
                if d < 0:
                    qlo, n = 0, QT
                else:
                    qlo, n = 128 * d, QT - 128 * d
                # head B always in the second psum bank: two concurrent
                # row-tiled matmuls writing one bank crashes the runtime
                boff = QT
                sp = scps.tile([128, 2 * QT], F32, name="sp", tag="sp")
                pt = att.tile([128, 2 * QT], BF, name="pt", tag="pt")
                for h in range(2):
                    r0 = h * 64
                    off = h * boff
                    nc.tensor.matmul(
                        sp[:, off:off + n],
                        kt_sb[p][r0:r0 + 64, kb * KB:(kb + 1) * KB],
                        qt_sb[p][r0:r0 + 64, q0 + qlo:q0 + QT],
                        start=True, stop=True)
                if boff == n or n == QT:
                    nc.scalar.activation(
                        pt[:, 0:boff + n], sp[:, 0:boff + n],
                        mybir.ActivationFunctionType.Exp, scale=float(SCALE))
                else:
                    for h in range(2):
                        off = h * boff
                        nc.scalar.activation(
                            pt[:, off:off + n], sp[:, off:off + n],
                            mybir.ActivationFunctionType.Exp, scale=float(SCALE))
                if d >= 0:
                    # diagonal triangle on the first 128 valid columns
                    nc.vector.tensor_mul(pt[:, 0:KB], pt[:, 0:KB], tri_sb[:])
                    nc.vector.tensor_mul(
                        pt[:, boff:boff + KB], pt[:, boff:boff + KB], tri_sb[:])
                for h in range(2):
                    hl = 2 * p + h
                    off = h * boff
                    nc.tensor.matmul(
                        cps[h][:, qlo:QT],
                        vt_sb[kb][:, hl * (DH + 1):(hl + 1) * (DH + 1)],
                        pt[:, off:off + n],
                        start=(kb == 0), stop=(kb == nkb - 1))
            for h in range(2):
                r0 = h * 64
                # l to SBUF via ACT Copy (custom-DVE recip_approx_fast gives
                # garbage reading PSUM directly; table-free, keeps DVE clear),
                # then fast approx reciprocal (~0.7us vs 3.35us plain)
                l_sb = attsm.tile([1, QT], F32, name="l_sb", tag="l")
                nc.scalar.copy(l_sb[:], cps[h][DH:DH + 1, :])
                r_sb = attsm.tile([1, QT], F32, name="r_sb", tag="r")
                nc.vector.reciprocal_approx_fast(r_sb[:], l_sb[:])
                rb = attsm.tile([64, QT], F32, name="rb", tag="rb")
                nc.gpsimd.partition_broadcast(rb[:], r_sb[:])
                nc.vector.tensor_mul(
                    ctx_sb[p][r0:r0 + 64, q0:q0 + QT], cps[h][0:DH, :], rb[:])

        def outproj(qt_i, ph3ps, ph3sb):
            """partial out-projection rows for one q tile; bias is added on
            the host after the cross-core partial sum."""
            for qb in range(qt_i * 4, qt_i * 4 + 4):
                os_ = ph3sb.tile([128, D], BF, name="os", tag="os")
                for nh in range(2):
                    op = ph3ps.tile([128, 512], F32, name="op", tag="op")
                    for p in range(PAIRS):
                        nc.tensor.matmul(
                            op[:], ctx_sb[p][:, qb * 128:(qb + 1) * 128],
                            wo_sb[p][:, nh * 512:(nh + 1) * 512],
                            start=(p == 0), stop=(p == PAIRS - 1))
                    nc.vector.tensor_copy(os_[:, nh * 512:(nh + 1) * 512], op[:])
                nc.sync.dma_start(out[qb * 128:(qb + 1) * 128, :], os_[:])

        # phase A: q/k pair 0, chunk-pipelined against the input DMAs
        with tc.tile_pool(name="qk0ps", bufs=1, space="PSUM") as qk0ps:
            proj_qk_chunked(0, qk0ps)

        # phase B onwards: V (2 psum banks) + attention pools (6 banks)
        with tc.tile_pool(name="att", bufs=4) as att, \
             tc.tile_pool(name="attsm", bufs=4) as attsm, \
             tc.tile_pool(name="scps", bufs=2, space="PSUM") as scps, \
             tc.tile_pool(name="ctxps", bufs=1, space="PSUM") as ctxps:

            def v_blocks(lo, hi, vps):
                for j in range(lo, hi):
                    vp = vps.tile([128, HPC * DH], F32, name="vp", tag="vp")
                    for i in range(DC):
                        nc.tensor.matmul(
                            vp[:], xts[i][:, j * 128:(j + 1) * 128], wv_sb[i][:],
                            start=(i == 0), stop=(i == DC - 1))
                    vt_view = vt_sb[j].rearrange("p (h e) -> p h e", h=HPC)
                    nc.vector.tensor_copy(
                        vt_view[:, :, 0:DH], vp.rearrange("p (h e) -> p h e", h=HPC))
                    nc.gpsimd.memset(vt_view[:, :, DH:DH + 1], 1.0)

            # pair-0 attention with the V blocks for q-tile t+1 emitted between
            # q-tiles: independent PE filler positioned at the cps-reuse stalls
            with tc.tile_pool(name="vps", bufs=2, space="PSUM") as vps:
                v_blocks(0, 4, vps)
                for qt_i in range(NQT):
                    attention_pair(0, qt_i, scps, att, ctxps, attsm)
                    if qt_i < NQT - 1:
                        v_blocks(4 * qt_i + 4, 4 * qt_i + 8, vps)

            # q/k pair 1 fills the tail of pair-0 attention
            with tc.tile_pool(name="qk1ps", bufs=2, space="PSUM") as qk1ps:
                proj_qk_seq(1, qk1ps)

            # pair-1 attention; outproj(qt-1) is fully ready by the end of
            # attention qt, so it fills the qt -> qt+1 boundary stall
            with tc.tile_pool(name="ph3ps", bufs=2, space="PSUM") as ph3ps, \
                 tc.tile_pool(name="ph3sb", bufs=3) as ph3sb:
                for qt_i in range(NQT):
                    attention_pair(1, qt_i, scps, att, ctxps, attsm)
                    if qt_i > 0:
                        outproj(qt_i - 1, ph3ps, ph3sb)
                outproj(NQT - 1, ph3ps, ph3sb)

    nc.compile()
    return nc


_NC = None
PROFILE = False
TRACE_CORES = (0,)
LAST_RESULT = None


def _get_nc():
    global _NC
    if _NC is None:
        _NC = _build()
    return _NC


def kernel(x, Wq, Wk, Wv, Wo, bo):
    x = np.asarray(x, dtype=np.float32)
    Wq = np.asarray(Wq, dtype=np.float32)
    Wk = np.asarray(Wk, dtype=np.float32)
    Wv = np.asarray(Wv, dtype=np.float32)
    Wo = np.asarray(Wo, dtype=np.float32)
    bo = np.asarray(bo, dtype=np.float32)

    nc = _get_nc()

    in_maps = _prepare_in_maps(x, Wq, Wk, Wv, Wo)

    global LAST_RESULT
    kw = {}
    if PROFILE:
        kw = dict(trace=True, trace_cores=list(TRACE_CORES))
    res = run_bass_kernel_spmd(nc, in_maps, core_ids=list(range(NCORES)), **kw)
    LAST_RESULT = res

    out = np.zeros((B, S, D), np.float32)
    for c in range(NCORES):
        b = c // 4
        out[b] += np.asarray(res.results[c]["out"], dtype=np.float32)
    out += bo[None, None, :]
    return out


def _prepare_in_maps(x, Wq, Wk, Wv, Wo):
    kk = np.arange(KB)[:, None]
    qq = np.arange(KB)[None, :]
    import ml_dtypes
    tri = (kk <= qq).astype(ml_dtypes.bfloat16)

    bf16 = ml_dtypes.bfloat16
    xTs = [np.ascontiguousarray(x[b].T).astype(bf16) for b in range(B)]

    in_maps = []
    for c in range(NCORES):
        b, g = divmod(c, 4)
        cs = slice(g * HPC * DH, (g + 1) * HPC * DH)
        in_maps.append({
            "xT": xTs[b],
            "wq": np.ascontiguousarray(Wq[:, cs]).astype(bf16),
            "wk": np.ascontiguousarray(Wk[:, cs]).astype(bf16),
            "wv": np.ascontiguousarray(Wv[:, cs]).astype(bf16),
            "wo": np.ascontiguousarray(Wo[cs, :]).astype(bf16),
            "tri": tri,
        })
    return in_maps


# revision 22
# speedup vs baseline: 1.2046x; 1.0964x over previous
"""Multi-head causal attention (B=2, S=2048, D=1024, H=16) on 8 trn2 cores.

Sharding: core c -> batch b=c//4, head-group g=c%4 (heads 4g..4g+3).
Each core: Q/K/V projections for its heads from xT[b], causal attention in
transposed layout, row-parallel out-projection partial in bf16. Host sums the
4 partials per batch and adds the bias.

Attention processes a head PAIR per call with row-tiled scores matmuls
(head A on PE rows 0-63, head B on rows 64-127) so the two scores streams
overlap and LDWEIGHTS hides under the other head's matmul. Diagonal
(straddle) blocks compute only the causally-valid q columns
(N = 512/384/256/128 for d = 0..3), so no memset masking is needed — only
the [128,128] triangle multiply on the first 128 valid columns.
"""

import numpy as np

import concourse.bass as bass
import concourse.tile as tile
import concourse.mybir as mybir
from concourse import bacc
from concourse.bass_utils import run_bass_kernel_spmd

B, S, D, H, DH = 2, 2048, 1024, 16, 64
NCORES = 8
HPC = 4          # heads per core
PAIRS = 2        # head pairs per core
QT = 512         # q tile (free dim of scoresT / PV matmuls)
KB = 128         # k block (partition dim of scoresT)
NQT = S // QT    # 4
NKB = S // KB    # 16
DC = D // 128    # 8 contraction chunks for projections
SCALE = 1.0 / np.sqrt(DH)
NWARM = 12       # junk matmuls to pre-warm the PE HAM during input DMA

F32 = mybir.dt.float32
BF = mybir.dt.bfloat16


def _build():
    nc = bacc.Bacc("TRN2", target_bir_lowering=False, debug=False, num_devices=NCORES)

    xT = nc.dram_tensor("xT", [D, S], BF, kind="ExternalInput").ap()
    wq = nc.dram_tensor("wq", [D, HPC * DH], BF, kind="ExternalInput").ap()
    wk = nc.dram_tensor("wk", [D, HPC * DH], BF, kind="ExternalInput").ap()
    wv = nc.dram_tensor("wv", [D, HPC * DH], BF, kind="ExternalInput").ap()
    wo = nc.dram_tensor("wo", [HPC * DH, D], BF, kind="ExternalInput").ap()
    tri = nc.dram_tensor("tri", [KB, KB], BF, kind="ExternalInput").ap()
    out = nc.dram_tensor("out", [S, D], BF, kind="ExternalOutput").ap()

    with tile.TileContext(nc) as tc, \
         tc.tile_pool(name="persist", bufs=1) as persist:
        # ---- persistent tiles ----
        qt_sb = [persist.tile([128, S], BF, name=f"qt{p}", tag=f"qt{p}") for p in range(PAIRS)]
        kt_sb = [persist.tile([128, S], BF, name=f"kt{p}", tag=f"kt{p}") for p in range(PAIRS)]
        # V' tiles: per s-block j, [128, 4*65]; head hl at cols 65*hl, ones col at 65*hl+64
        vt_sb = [persist.tile([128, HPC * (DH + 1)], BF, name=f"vt{j}", tag=f"vt{j}") for j in range(NKB)]
        ctx_sb = [persist.tile([128, S], BF, name=f"ctx{p}", tag=f"ctx{p}") for p in range(PAIRS)]
        wo_sb = [persist.tile([128, D], BF, name=f"wo{p}", tag=f"wo{p}") for p in range(PAIRS)]
        tri_sb = persist.tile([KB, KB], BF, name="tri", tag="tri")
        junk_sb = persist.tile([128, QT], BF, name="junk", tag="junk")

        xts = [persist.tile([128, S], BF, name=f"xts{i}", tag=f"xts{i}") for i in range(DC)]
        wq_sb = [persist.tile([128, HPC * DH], BF, name=f"wq{i}", tag=f"wq{i}") for i in range(DC)]
        wk_sb = [persist.tile([128, HPC * DH], BF, name=f"wk{i}", tag=f"wk{i}") for i in range(DC)]
        wv_sb = [persist.tile([128, HPC * DH], BF, name=f"wv{i}", tag=f"wv{i}") for i in range(DC)]

        nc.gpsimd.memset(junk_sb[:], 0.0)
        # chunk-major DMA order so projection chunk i can chase chunk-i DMAs;
        # xT chunks split into S/4 quarters so the first matmul starts ~1us in
        for i in range(DC):
            if i == 0:
                nc.sync.dma_start(wq_sb[i][:], wq[i * 128:(i + 1) * 128, :])
            for st in range(NQT):
                nc.sync.dma_start(xts[i][:, st * QT:(st + 1) * QT],
                                  xT[i * 128:(i + 1) * 128, st * QT:(st + 1) * QT])
            if i > 0:
                nc.sync.dma_start(wq_sb[i][:], wq[i * 128:(i + 1) * 128, :])
            nc.sync.dma_start(wk_sb[i][:], wk[i * 128:(i + 1) * 128, :])
            nc.sync.dma_start(wv_sb[i][:], wv[i * 128:(i + 1) * 128, :])
            if i == 0:
                nc.sync.dma_start(tri_sb[:], tri[:])
        for p in range(PAIRS):
            nc.sync.dma_start(wo_sb[p][:], wo[p * 128:(p + 1) * 128, :])

        # HAM warmup: junk matmuls with no input deps keep the PE busy while
        # the input DMAs land, so the projections start at full clock
        with tc.tile_pool(name="warmps", bufs=1, space="PSUM") as warmps:
            wp = warmps.tile([128, QT], F32, name="wp", tag="wp")
            for _ in range(NWARM):
                nc.tensor.matmul(wp[:], junk_sb[:, 0:128], junk_sb[:],
                                 start=True, stop=True)

        def proj_qk_chunked(p, pool):
            """q/k projection for pair p, D-chunk-outer so matmuls chase the
            xT DMAs chunk by chunk. Holds 8 psum banks."""
            qps = [pool.tile([128, QT], F32, name=f"qps{st}", tag=f"qk{st}") for st in range(NQT)]
            kps = [pool.tile([128, QT], F32, name=f"kps{st}", tag=f"qk{4 + st}") for st in range(NQT)]
            for i in range(DC):
                for st in range(NQT):
                    nc.tensor.matmul(
                        qps[st][:], wq_sb[i][:, p * 128:(p + 1) * 128],
                        xts[i][:, st * QT:(st + 1) * QT],
                        start=(i == 0), stop=(i == DC - 1))
                for st in range(NQT):
                    nc.tensor.matmul(
                        kps[st][:], wk_sb[i][:, p * 128:(p + 1) * 128],
                        xts[i][:, st * QT:(st + 1) * QT],
                        start=(i == 0), stop=(i == DC - 1))
            for st in range(NQT):
                nc.vector.tensor_copy(qt_sb[p][:, st * QT:(st + 1) * QT], qps[st][:])
                nc.vector.tensor_copy(kt_sb[p][:, st * QT:(st + 1) * QT], kps[st][:])

        def proj_qk_seq(p, pool):
            """q/k projection, sequential psum (2 banks) — overlaps with
            attention of the other pair."""
            for st in range(NQT):
                qp = pool.tile([128, QT], F32, name="qp", tag="qkseq")
                for i in range(DC):
                    nc.tensor.matmul(
                        qp[:], wq_sb[i][:, p * 128:(p + 1) * 128],
                        xts[i][:, st * QT:(st + 1) * QT],
                        start=(i == 0), stop=(i == DC - 1))
                nc.vector.tensor_copy(qt_sb[p][:, st * QT:(st + 1) * QT], qp[:])
                kp = pool.tile([128, QT], F32, name="kp", tag="qkseq")
                for i in range(DC):
                    nc.tensor.matmul(
                        kp[:], wk_sb[i][:, p * 128:(p + 1) * 128],
                        xts[i][:, st * QT:(st + 1) * QT],
                        start=(i == 0), stop=(i == DC - 1))
                nc.vector.tensor_copy(kt_sb[p][:, st * QT:(st + 1) * QT], kp[:])

        def attention_pair(p, qt_i, scps, att, ctxps, attsm):
            """Both heads of pair p for q-tile qt_i. Scores row-tiled: head A
            contracts on PE rows 0-63, head B on rows 64-127 (base_partition
            derives tile_position), so the two streams overlap and LDW hides.
            Straddle blocks d=0..3 compute only q columns >= 128*d."""
            q0 = qt_i * QT
            nkb = 4 * qt_i + 4
            cps = [ctxps.tile([DH + 1, QT], F32, name=f"cps{h}", tag=f"cps{h}")
                   for h in range(2)]

            def geom(kb):
                d = kb - 4 * qt_i
                qlo = 0 if d < 0 else 128 * d
                return qlo, QT - qlo

            # k blocks in groups of two, so the two PV matmuls per head chain
            # into the same psum bank (bank-alternating MMs pay ~+100ns each)
            for g0 in range(0, nkb, 2):
                sps, pts = [], []
                for u in range(2):
                    kb = g0 + u
                    qlo, n = geom(kb)
                    # head B always in the second psum bank: two concurrent
                    # row-tiled matmuls writing one bank crashes the runtime
                    sp = scps.tile([128, 2 * QT], F32, name="sp", tag="sp")
                    pt = att.tile([128, 2 * QT], BF, name="pt", tag="pt")
                    sps.append(sp)
                    pts.append(pt)
                    for h in range(2):
                        r0 = h * 64
                        off = h * QT
                        nc.tensor.matmul(
                            sp[:, off:off + n],
                            kt_sb[p][r0:r0 + 64, kb * KB:(kb + 1) * KB],
                            qt_sb[p][r0:r0 + 64, q0 + qlo:q0 + QT],
                            start=True, stop=True)
                for u in range(2):
                    kb = g0 + u
                    qlo, n = geom(kb)
                    sp, pt = sps[u], pts[u]
                    if n == QT:
                        nc.scalar.activation(
                            pt[:, 0:QT + n], sp[:, 0:QT + n],
                            mybir.ActivationFunctionType.Exp, scale=float(SCALE))
                    else:
                        for h in range(2):
                            off = h * QT
                            nc.scalar.activation(
                                pt[:, off:off + n], sp[:, off:off + n],
                                mybir.ActivationFunctionType.Exp,
                                scale=float(SCALE))
                    if kb >= 4 * qt_i:
                        # diagonal triangle on the first 128 valid columns
                        nc.vector.tensor_mul(pt[:, 0:KB], pt[:, 0:KB], tri_sb[:])
                        nc.vector.tensor_mul(
                            pt[:, QT:QT + KB], pt[:, QT:QT + KB], tri_sb[:])
                for h in range(2):
                    hl = 2 * p + h
                    off = h * QT
                    for u in range(2):
                        kb = g0 + u
                        qlo, n = geom(kb)
                        nc.tensor.matmul(
                            cps[h][:, qlo:QT],
                            vt_sb[kb][:, hl * (DH + 1):(hl + 1) * (DH + 1)],
                            pts[u][:, off:off + n],
                            start=(kb == 0), stop=(kb == nkb - 1))
            for h in range(2):
                r0 = h * 64
                # l to SBUF via ACT Copy (custom-DVE recip_approx_fast gives
                # garbage reading PSUM directly; table-free, keeps DVE clear),
                # then fast approx reciprocal (~0.7us vs 3.35us plain)
                l_sb = attsm.tile([1, QT], F32, name="l_sb", tag="l")
                nc.scalar.copy(l_sb[:], cps[h][DH:DH + 1, :])
                r_sb = attsm.tile([1, QT], F32, name="r_sb", tag="r")
                nc.vector.reciprocal_approx_fast(r_sb[:], l_sb[:])
                rb = attsm.tile([64, QT], F32, name="rb", tag="rb")
                nc.gpsimd.partition_broadcast(rb[:], r_sb[:])
                nc.vector.tensor_mul(
                    ctx_sb[p][r0:r0 + 64, q0:q0 + QT], cps[h][0:DH, :], rb[:])

        def outproj(qt_i, ph3ps, ph3sb):
            """partial out-projection rows for one q tile; bias is added on
            the host after the cross-core partial sum."""
            for qb in range(qt_i * 4, qt_i * 4 + 4):
                os_ = ph3sb.tile([128, D], BF, name="os", tag="os")
                for nh in range(2):
                    op = ph3ps.tile([128, 512], F32, name="op", tag="op")
                    for p in range(PAIRS):
                        nc.tensor.matmul(
                            op[:], ctx_sb[p][:, qb * 128:(qb + 1) * 128],
                            wo_sb[p][:, nh * 512:(nh + 1) * 512],
                            start=(p == 0), stop=(p == PAIRS - 1))
                    nc.vector.tensor_copy(os_[:, nh * 512:(nh + 1) * 512], op[:])
                nc.sync.dma_start(out[qb * 128:(qb + 1) * 128, :], os_[:])

        # phase A: q/k pair 0, chunk-pipelined against the input DMAs
        with tc.tile_pool(name="qk0ps", bufs=1, space="PSUM") as qk0ps:
            proj_qk_chunked(0, qk0ps)

        # phase B onwards: V (2 psum banks) + attention pools (6 banks)
        with tc.tile_pool(name="att", bufs=4) as att, \
             tc.tile_pool(name="attsm", bufs=4) as attsm, \
             tc.tile_pool(name="scps", bufs=2, space="PSUM") as scps, \
             tc.tile_pool(name="ctxps", bufs=1, space="PSUM") as ctxps:

            def v_blocks(lo, hi, vps):
                for j in range(lo, hi):
                    vp = vps.tile([128, HPC * DH], F32, name="vp", tag="vp")
                    for i in range(DC):
                        nc.tensor.matmul(
                            vp[:], xts[i][:, j * 128:(j + 1) * 128], wv_sb[i][:],
                            start=(i == 0), stop=(i == DC - 1))
                    vt_view = vt_sb[j].rearrange("p (h e) -> p h e", h=HPC)
                    nc.vector.tensor_copy(
                        vt_view[:, :, 0:DH], vp.rearrange("p (h e) -> p h e", h=HPC))
                    nc.gpsimd.memset(vt_view[:, :, DH:DH + 1], 1.0)

            # pair-0 attention with the V blocks for q-tile t+1 emitted between
            # q-tiles: independent PE filler positioned at the cps-reuse stalls
            with tc.tile_pool(name="vps", bufs=2, space="PSUM") as vps:
                v_blocks(0, 4, vps)
                for qt_i in range(NQT):
                    attention_pair(0, qt_i, scps, att, ctxps, attsm)
                    if qt_i < NQT - 1:
                        v_blocks(4 * qt_i + 4, 4 * qt_i + 8, vps)

            # q/k pair 1 fills the tail of pair-0 attention
            with tc.tile_pool(name="qk1ps", bufs=2, space="PSUM") as qk1ps:
                proj_qk_seq(1, qk1ps)

            # pair-1 attention; outproj(qt-1) is fully ready by the end of
            # attention qt, so it fills the qt -> qt+1 boundary stall
            with tc.tile_pool(name="ph3ps", bufs=2, space="PSUM") as ph3ps, \
                 tc.tile_pool(name="ph3sb", bufs=3) as ph3sb:
                for qt_i in range(NQT):
                    attention_pair(1, qt_i, scps, att, ctxps, attsm)
                    if qt_i > 0:
                        outproj(qt_i - 1, ph3ps, ph3sb)
                outproj(NQT - 1, ph3ps, ph3sb)

    nc.compile()
    return nc


_NC = None
PROFILE = False
TRACE_CORES = (0,)
LAST_RESULT = None


def _get_nc():
    global _NC
    if _NC is None:
        _NC = _build()
    return _NC


def kernel(x, Wq, Wk, Wv, Wo, bo):
    x = np.asarray(x, dtype=np.float32)
    Wq = np.asarray(Wq, dtype=np.float32)
    Wk = np.asarray(Wk, dtype=np.float32)
    Wv = np.asarray(Wv, dtype=np.float32)
    Wo = np.asarray(Wo, dtype=np.float32)
    bo = np.asarray(bo, dtype=np.float32)

    nc = _get_nc()

    in_maps = _prepare_in_maps(x, Wq, Wk, Wv, Wo)

    global LAST_RESULT
    kw = {}
    if PROFILE:
        kw = dict(trace=True, trace_cores=list(TRACE_CORES))
    res = run_bass_kernel_spmd(nc, in_maps, core_ids=list(range(NCORES)), **kw)
    LAST_RESULT = res

    out = np.zeros((B, S, D), np.float32)
    for c in range(NCORES):
        b = c // 4
        out[b] += np.asarray(res.results[c]["out"], dtype=np.float32)
    out += bo[None, None, :]
    return out


def _prepare_in_maps(x, Wq, Wk, Wv, Wo):
    kk = np.arange(KB)[:, None]
    qq = np.arange(KB)[None, :]
    import ml_dtypes
    tri = (kk <= qq).astype(ml_dtypes.bfloat16)

    bf16 = ml_dtypes.bfloat16
    xTs = [np.ascontiguousarray(x[b].T).astype(bf16) for b in range(B)]

    in_maps = []
    for c in range(NCORES):
        b, g = divmod(c, 4)
        cs = slice(g * HPC * DH, (g + 1) * HPC * DH)
        in_maps.append({
            "xT": xTs[b],
            "wq": np.ascontiguousarray(Wq[:, cs]).astype(bf16),
            "wk": np.ascontiguousarray(Wk[:, cs]).astype(bf16),
            "wv": np.ascontiguousarray(Wv[:, cs]).astype(bf16),
            "wo": np.ascontiguousarray(Wo[cs, :]).astype(bf16),
            "tri": tri,
        })
    return in_maps


# revision 24
# speedup vs baseline: 1.3075x; 1.0854x over previous
"""Multi-head causal attention (B=2, S=2048, D=1024, H=16) on 8 trn2 cores.

Sharding: core c -> batch b=c//4, head-group g=c%4 (heads 4g..4g+3).
Each core: Q/K/V projections for its heads from xT[b], causal attention in
transposed layout, row-parallel out-projection partial in bf16. Host sums the
4 partials per batch and adds the bias.

Attention processes a head PAIR per call with row-tiled scores matmuls
(head A on PE rows 0-63, head B on rows 64-127) so the two scores streams
overlap and LDWEIGHTS hides under the other head's matmul. Diagonal
(straddle) blocks compute only the causally-valid q columns
(N = 512/384/256/128 for d = 0..3), so no memset masking is needed — only
the [128,128] triangle multiply on the first 128 valid columns.
"""

import numpy as np

import concourse.bass as bass
import concourse.tile as tile
import concourse.mybir as mybir
from concourse import bacc
from concourse.bass_utils import run_bass_kernel_spmd

B, S, D, H, DH = 2, 2048, 1024, 16, 64
NCORES = 8
HPC = 4          # heads per core
PAIRS = 2        # head pairs per core
QT = 512         # q tile (free dim of scoresT / PV matmuls)
KB = 128         # k block (partition dim of scoresT)
NQT = S // QT    # 4
NKB = S // KB    # 16
DC = D // 128    # 8 contraction chunks for projections
SCALE = 1.0 / np.sqrt(DH)
NWARM = 8        # junk matmuls to pre-warm the PE HAM during input DMA

F32 = mybir.dt.float32
BF = mybir.dt.bfloat16


def _build():
    nc = bacc.Bacc("TRN2", target_bir_lowering=False, debug=False, num_devices=NCORES)

    xT = nc.dram_tensor("xT", [D, S], BF, kind="ExternalInput").ap()
    wq = nc.dram_tensor("wq", [D, HPC * DH], BF, kind="ExternalInput").ap()
    wk = nc.dram_tensor("wk", [D, HPC * DH], BF, kind="ExternalInput").ap()
    wv = nc.dram_tensor("wv", [D, HPC * DH], BF, kind="ExternalInput").ap()
    wo = nc.dram_tensor("wo", [HPC * DH, D], BF, kind="ExternalInput").ap()
    tri = nc.dram_tensor("tri", [KB, KB], BF, kind="ExternalInput").ap()
    out = nc.dram_tensor("out", [S, D], BF, kind="ExternalOutput").ap()

    with tile.TileContext(nc) as tc, \
         tc.tile_pool(name="persist", bufs=1) as persist:
        # ---- persistent tiles ----
        qt_sb = [persist.tile([128, S], BF, name=f"qt{p}", tag=f"qt{p}") for p in range(PAIRS)]
        kt_sb = [persist.tile([128, S], BF, name=f"kt{p}", tag=f"kt{p}") for p in range(PAIRS)]
        # V' tiles: per s-block j, [128, 4*65]; head hl at cols 65*hl, ones col at 65*hl+64
        vt_sb = [persist.tile([128, HPC * (DH + 1)], BF, name=f"vt{j}", tag=f"vt{j}") for j in range(NKB)]
        ctx_sb = [persist.tile([128, S], BF, name=f"ctx{p}", tag=f"ctx{p}") for p in range(PAIRS)]
        wo_sb = [persist.tile([128, D], BF, name=f"wo{p}", tag=f"wo{p}") for p in range(PAIRS)]
        tri_sb = persist.tile([KB, KB], BF, name="tri", tag="tri")
        junk_sb = persist.tile([128, QT], BF, name="junk", tag="junk")

        xts = [persist.tile([128, S], BF, name=f"xts{i}", tag=f"xts{i}") for i in range(DC)]
        wq_sb = [persist.tile([128, HPC * DH], BF, name=f"wq{i}", tag=f"wq{i}") for i in range(DC)]
        wk_sb = [persist.tile([128, HPC * DH], BF, name=f"wk{i}", tag=f"wk{i}") for i in range(DC)]
        wv_sb = [persist.tile([128, HPC * DH], BF, name=f"wv{i}", tag=f"wv{i}") for i in range(DC)]

        nc.gpsimd.memset(junk_sb[:], 0.0)
        # chunk-major DMA order so projection chunk i can chase chunk-i DMAs.
        # xT rides the sync HWDGE ring; weights ride the scalar HWDGE ring and
        # wo/tri the gpsimd SWDGE, so the three streams drain in parallel.
        for i in range(DC):
            nc.sync.dma_start(xts[i][:], xT[i * 128:(i + 1) * 128, :])
            nc.scalar.dma_start(wq_sb[i][:], wq[i * 128:(i + 1) * 128, :])
            nc.scalar.dma_start(wk_sb[i][:], wk[i * 128:(i + 1) * 128, :])
            nc.scalar.dma_start(wv_sb[i][:], wv[i * 128:(i + 1) * 128, :])
        nc.gpsimd.dma_start(tri_sb[:], tri[:])
        for p in range(PAIRS):
            nc.gpsimd.dma_start(wo_sb[p][:], wo[p * 128:(p + 1) * 128, :])

        # HAM warmup: junk matmuls with no input deps keep the PE busy while
        # the input DMAs land, so the projections start at full clock
        with tc.tile_pool(name="warmps", bufs=1, space="PSUM") as warmps:
            wp = warmps.tile([128, QT], F32, name="wp", tag="wp")
            for _ in range(NWARM):
                nc.tensor.matmul(wp[:], junk_sb[:, 0:128], junk_sb[:],
                                 start=True, stop=True)

        def proj_qk_chunked(p, pool):
            """q/k projection for pair p, D-chunk-outer so matmuls chase the
            xT DMAs chunk by chunk. Holds 8 psum banks."""
            qps = [pool.tile([128, QT], F32, name=f"qps{st}", tag=f"qk{st}") for st in range(NQT)]
            kps = [pool.tile([128, QT], F32, name=f"kps{st}", tag=f"qk{4 + st}") for st in range(NQT)]
            for i in range(DC):
                for st in range(NQT):
                    nc.tensor.matmul(
                        qps[st][:], wq_sb[i][:, p * 128:(p + 1) * 128],
                        xts[i][:, st * QT:(st + 1) * QT],
                        start=(i == 0), stop=(i == DC - 1))
                for st in range(NQT):
                    nc.tensor.matmul(
                        kps[st][:], wk_sb[i][:, p * 128:(p + 1) * 128],
                        xts[i][:, st * QT:(st + 1) * QT],
                        start=(i == 0), stop=(i == DC - 1))
            for st in range(NQT):
                nc.vector.tensor_copy(qt_sb[p][:, st * QT:(st + 1) * QT], qps[st][:])
                nc.vector.tensor_copy(kt_sb[p][:, st * QT:(st + 1) * QT], kps[st][:])

        def proj_qk_seq(p, pool):
            """q/k projection, sequential psum (2 banks) — overlaps with
            attention of the other pair."""
            for st in range(NQT):
                qp = pool.tile([128, QT], F32, name="qp", tag="qkseq")
                for i in range(DC):
                    nc.tensor.matmul(
                        qp[:], wq_sb[i][:, p * 128:(p + 1) * 128],
                        xts[i][:, st * QT:(st + 1) * QT],
                        start=(i == 0), stop=(i == DC - 1))
                nc.vector.tensor_copy(qt_sb[p][:, st * QT:(st + 1) * QT], qp[:])
                kp = pool.tile([128, QT], F32, name="kp", tag="qkseq")
                for i in range(DC):
                    nc.tensor.matmul(
                        kp[:], wk_sb[i][:, p * 128:(p + 1) * 128],
                        xts[i][:, st * QT:(st + 1) * QT],
                        start=(i == 0), stop=(i == DC - 1))
                nc.vector.tensor_copy(kt_sb[p][:, st * QT:(st + 1) * QT], kp[:])

        def attention_pair(p, qt_i, scps, att, ctxps, attsm):
            """Both heads of pair p for q-tile qt_i. Scores row-tiled: head A
            contracts on PE rows 0-63, head B on rows 64-127 (base_partition
            derives tile_position), so the two streams overlap and LDW hides.
            Straddle blocks d=0..3 compute only q columns >= 128*d."""
            q0 = qt_i * QT
            nkb = 4 * qt_i + 4
            cps = [ctxps.tile([DH + 1, QT], F32, name=f"cps{h}", tag=f"cps{h}")
                   for h in range(2)]

            def geom(kb):
                d = kb - 4 * qt_i
                qlo = 0 if d < 0 else 128 * d
                return qlo, QT - qlo

            # k blocks in groups of two, so the two PV matmuls per head chain
            # into the same psum bank (bank-alternating MMs pay ~+100ns each)
            for g0 in range(0, nkb, 2):
                sps, pts = [], []
                for u in range(2):
                    kb = g0 + u
                    qlo, n = geom(kb)
                    # head B always in the second psum bank: two concurrent
                    # row-tiled matmuls writing one bank crashes the runtime
                    sp = scps.tile([128, 2 * QT], F32, name="sp", tag="sp")
                    pt = att.tile([128, 2 * QT], BF, name="pt", tag="pt")
                    sps.append(sp)
                    pts.append(pt)
                    for h in range(2):
                        r0 = h * 64
                        off = h * QT
                        nc.tensor.matmul(
                            sp[:, off:off + n],
                            kt_sb[p][r0:r0 + 64, kb * KB:(kb + 1) * KB],
                            qt_sb[p][r0:r0 + 64, q0 + qlo:q0 + QT],
                            start=True, stop=True)
                for u in range(2):
                    kb = g0 + u
                    qlo, n = geom(kb)
                    sp, pt = sps[u], pts[u]
                    if n == QT:
                        nc.scalar.activation(
                            pt[:, 0:QT + n], sp[:, 0:QT + n],
                            mybir.ActivationFunctionType.Exp, scale=float(SCALE))
                    else:
                        for h in range(2):
                            off = h * QT
                            nc.scalar.activation(
                                pt[:, off:off + n], sp[:, off:off + n],
                                mybir.ActivationFunctionType.Exp,
                                scale=float(SCALE))
                    if kb >= 4 * qt_i:
                        # diagonal triangle on the first 128 valid columns
                        nc.vector.tensor_mul(pt[:, 0:KB], pt[:, 0:KB], tri_sb[:])
                        nc.vector.tensor_mul(
                            pt[:, QT:QT + KB], pt[:, QT:QT + KB], tri_sb[:])
                for h in range(2):
                    hl = 2 * p + h
                    off = h * QT
                    for u in range(2):
                        kb = g0 + u
                        qlo, n = geom(kb)
                        nc.tensor.matmul(
                            cps[h][:, qlo:QT],
                            vt_sb[kb][:, hl * (DH + 1):(hl + 1) * (DH + 1)],
                            pts[u][:, off:off + n],
                            start=(kb == 0), stop=(kb == nkb - 1))
            for h in range(2):
                r0 = h * 64
                # l to SBUF via ACT Copy (custom-DVE recip_approx_fast gives
                # garbage reading PSUM directly; table-free, keeps DVE clear),
                # then fast approx reciprocal (~0.7us vs 3.35us plain)
                l_sb = attsm.tile([1, QT], F32, name="l_sb", tag="l")
                nc.scalar.copy(l_sb[:], cps[h][DH:DH + 1, :])
                r_sb = attsm.tile([1, QT], F32, name="r_sb", tag="r")
                nc.vector.reciprocal_approx_fast(r_sb[:], l_sb[:])
                rb = attsm.tile([64, QT], F32, name="rb", tag="rb")
                nc.gpsimd.partition_broadcast(rb[:], r_sb[:])
                nc.vector.tensor_mul(
                    ctx_sb[p][r0:r0 + 64, q0:q0 + QT], cps[h][0:DH, :], rb[:])

        def outproj(qt_i, ph3ps, ph3sb):
            """partial out-projection rows for one q tile; bias is added on
            the host after the cross-core partial sum."""
            for qb in range(qt_i * 4, qt_i * 4 + 4):
                os_ = ph3sb.tile([128, D], BF, name="os", tag="os")
                for nh in range(2):
                    op = ph3ps.tile([128, 512], F32, name="op", tag="op")
                    for p in range(PAIRS):
                        nc.tensor.matmul(
                            op[:], ctx_sb[p][:, qb * 128:(qb + 1) * 128],
                            wo_sb[p][:, nh * 512:(nh + 1) * 512],
                            start=(p == 0), stop=(p == PAIRS - 1))
                    nc.vector.tensor_copy(os_[:, nh * 512:(nh + 1) * 512], op[:])
                nc.sync.dma_start(out[qb * 128:(qb + 1) * 128, :], os_[:])

        # phase A: q/k pair 0, chunk-pipelined against the input DMAs
        with tc.tile_pool(name="qk0ps", bufs=1, space="PSUM") as qk0ps:
            proj_qk_chunked(0, qk0ps)

        # phase B onwards: V (2 psum banks) + attention pools (6 banks)
        with tc.tile_pool(name="att", bufs=4) as att, \
             tc.tile_pool(name="attsm", bufs=4) as attsm, \
             tc.tile_pool(name="scps", bufs=2, space="PSUM") as scps, \
             tc.tile_pool(name="ctxps", bufs=1, space="PSUM") as ctxps:

            def v_blocks(lo, hi, vps):
                for j in range(lo, hi):
                    vp = vps.tile([128, HPC * DH], F32, name="vp", tag="vp")
                    for i in range(DC):
                        nc.tensor.matmul(
                            vp[:], xts[i][:, j * 128:(j + 1) * 128], wv_sb[i][:],
                            start=(i == 0), stop=(i == DC - 1))
                    vt_view = vt_sb[j].rearrange("p (h e) -> p h e", h=HPC)
                    nc.vector.tensor_copy(
                        vt_view[:, :, 0:DH], vp.rearrange("p (h e) -> p h e", h=HPC))
                    nc.gpsimd.memset(vt_view[:, :, DH:DH + 1], 1.0)

            # pair-0 attention with the V blocks for q-tile t+1 emitted between
            # q-tiles: independent PE filler positioned at the cps-reuse stalls
            with tc.tile_pool(name="vps", bufs=2, space="PSUM") as vps:
                v_blocks(0, 4, vps)
                for qt_i in range(NQT):
                    attention_pair(0, qt_i, scps, att, ctxps, attsm)
                    if qt_i < NQT - 1:
                        v_blocks(4 * qt_i + 4, 4 * qt_i + 8, vps)

            # q/k pair 1 fills the tail of pair-0 attention
            with tc.tile_pool(name="qk1ps", bufs=2, space="PSUM") as qk1ps:
                proj_qk_seq(1, qk1ps)

            # pair-1 attention; outproj(qt-1) is fully ready by the end of
            # attention qt, so it fills the qt -> qt+1 boundary stall
            with tc.tile_pool(name="ph3ps", bufs=2, space="PSUM") as ph3ps, \
                 tc.tile_pool(name="ph3sb", bufs=3) as ph3sb:
                for qt_i in range(NQT):
                    attention_pair(1, qt_i, scps, att, ctxps, attsm)
                    if qt_i > 0:
                        outproj(qt_i - 1, ph3ps, ph3sb)
                outproj(NQT - 1, ph3ps, ph3sb)

    nc.compile()
    return nc


_NC = None
PROFILE = False
TRACE_CORES = (0,)
LAST_RESULT = None


def _get_nc():
    global _NC
    if _NC is None:
        _NC = _build()
    return _NC


def kernel(x, Wq, Wk, Wv, Wo, bo):
    x = np.asarray(x, dtype=np.float32)
    Wq = np.asarray(Wq, dtype=np.float32)
    Wk = np.asarray(Wk, dtype=np.float32)
    Wv = np.asarray(Wv, dtype=np.float32)
    Wo = np.asarray(Wo, dtype=np.float32)
    bo = np.asarray(bo, dtype=np.float32)

    nc = _get_nc()

    in_maps = _prepare_in_maps(x, Wq, Wk, Wv, Wo)

    global LAST_RESULT
    kw = {}
    if PROFILE:
        kw = dict(trace=True, trace_cores=list(TRACE_CORES))
    res = run_bass_kernel_spmd(nc, in_maps, core_ids=list(range(NCORES)), **kw)
    LAST_RESULT = res

    out = np.zeros((B, S, D), np.float32)
    for c in range(NCORES):
        b = c // 4
        out[b] += np.asarray(res.results[c]["out"], dtype=np.float32)
    out += bo[None, None, :]
    return out


def _prepare_in_maps(x, Wq, Wk, Wv, Wo):
    kk = np.arange(KB)[:, None]
    qq = np.arange(KB)[None, :]
    import ml_dtypes
    tri = (kk <= qq).astype(ml_dtypes.bfloat16)

    bf16 = ml_dtypes.bfloat16
    xTs = [np.ascontiguousarray(x[b].T).astype(bf16) for b in range(B)]

    in_maps = []
    for c in range(NCORES):
        b, g = divmod(c, 4)
        cs = slice(g * HPC * DH, (g + 1) * HPC * DH)
        in_maps.append({
            "xT": xTs[b],
            "wq": np.ascontiguousarray(Wq[:, cs]).astype(bf16),
            "wk": np.ascontiguousarray(Wk[:, cs]).astype(bf16),
            "wv": np.ascontiguousarray(Wv[:, cs]).astype(bf16),
            "wo": np.ascontiguousarray(Wo[cs, :]).astype(bf16),
            "tri": tri,
        })
    return in_maps
